# revision 4
# baseline (speedup 1.0000x reference)
"""Trainium2 Bass kernel for ConvSelfAttention (SAGAN-style 1x1-conv attention).

Per-batch math (b=8 batches, one per NeuronCore):
    x   = v.reshape(C, N)                 C=256, N=4096
    qkv = Wqkv @ x                        q,k,val each (64, N)
    s   = q^T k                           (N, N)
    beta = softmax(s, axis=1)             row softmax
    y   = val @ beta                      (64, N)
    o   = gamma * (Wout @ y) + x

Flash-style single-pass attention per 128-row i-chunk: s row-block is
computed on the PE into PSUM, exp'd on the scalar engine (with fused
row-sum accumulation for the softmax denominator), the reciprocal
denominator is folded into val^T, and y^T is accumulated in PSUM across
all i-chunks (contraction over i) without ever materializing the
(N, N) attention matrix.  Softmax max-subtraction is skipped: |s| < ~6
for this problem so exp() is well within fp32 range and the result is
mathematically identical.
"""

import sys

for _p in ("/opt/trn_rl_repo",):
    if _p not in sys.path:
        sys.path.insert(0, _p)

from contextlib import ExitStack

import numpy as np

import concourse.bass as bass
import concourse.bacc as bacc
import concourse.mybir as mybir
import concourse.tile as tile
from concourse.bass import ts
from concourse.bass_utils import run_bass_kernel_spmd
from concourse.masks import make_identity
from concourse.tile import add_dep_helper

BS, C, N, DK = 8, 256, 4096, 64
P = 128            # SBUF/PSUM partitions
JS = 512           # j stripe width (max fp32 matmul free dim / PSUM bank)
NJS = N // JS      # 8 stripes per row-block
NI = N // P        # 32 i-chunks (and j-chunks)
CPB = 2048 // (DK * 4)   # y^T chunks per PSUM bank = 8
DT = mybir.dt.float32
AX = mybir.AxisListType.X
EXP = mybir.ActivationFunctionType.Exp

_CACHED = {}


def _build_nc():
    nc = bacc.Bacc(None)
    x_d = nc.dram_tensor("x", [C, N], DT, kind="ExternalInput")
    wqkv_d = nc.dram_tensor("wqkv", [3 * DK, C], DT, kind="ExternalInput")
    wout_d = nc.dram_tensor("wout", [C, DK], DT, kind="ExternalInput")
    gamma_d = nc.dram_tensor("gamma", [1, 1], DT, kind="ExternalInput")
    o_d = nc.dram_tensor("o", [C, N], DT, kind="ExternalOutput")

    with tile.TileContext(nc) as tc, ExitStack() as ctx:
        singles = ctx.enter_context(tc.tile_pool(name="singles", bufs=1))
        big = ctx.enter_context(tc.tile_pool(name="big", bufs=1))
        e_pool = ctx.enter_context(tc.tile_pool(name="epool", bufs=2))
        small = ctx.enter_context(tc.tile_pool(name="small", bufs=2))
        outp = ctx.enter_context(tc.tile_pool(name="outp", bufs=3))
        ps_scr = ctx.enter_context(tc.tile_pool(name="ps_scr", bufs=2, space="PSUM"))
        ps_s = ctx.enter_context(tc.tile_pool(name="ps_s", bufs=2, space="PSUM"))
        ps_yt = ctx.enter_context(tc.tile_pool(name="ps_yt", bufs=1, space="PSUM"))

        ident = singles.tile([P, P], DT)
        make_identity(nc, ident)

        # ---- weights: load raw, transpose on PE into matmul-ready layouts
        wqk_raw = singles.tile([P, C], DT)        # Wqkv rows 0:128 = [Wq; Wk]
        wv_raw = singles.tile([DK, C], DT)        # Wqkv rows 128:192 = Wv
        wout_raw = singles.tile([P, 2, DK], DT)   # Wout (256, 64), 2 row-chunks
        gamma_t = singles.tile([DK, 1], DT)
        nc.sync.dma_start(out=wqk_raw, in_=wqkv_d[0:P, :])
        nc.sync.dma_start(out=wv_raw, in_=wqkv_d[P : 3 * DK, :])
        for oc in range(2):
            nc.sync.dma_start(out=wout_raw[:, oc, :], in_=wout_d[ts(oc, P), :])
        gd = gamma_d[:]
        nc.sync.dma_start(
            out=gamma_t,
            in_=bass.AP(tensor=gd.tensor, offset=gd.offset, ap=[[0, DK], [1, 1]]),
        )

        wqkT = singles.tile([P, 2, P], DT)        # (c-chunk part, ci, [q|k] out ch)
        wvT = singles.tile([P, 2, DK], DT)
        woutT = singles.tile([DK, C], DT)         # Wout^T, gamma folded in
        for ci in range(2):
            pt = ps_scr.tile([P, P], DT, tag="scr")
            nc.tensor.transpose(pt, wqk_raw[:, ts(ci, P)], ident)
            nc.vector.tensor_copy(wqkT[:, ci, :], pt)
            pv = ps_scr.tile([P, DK], DT, tag="scr")
            nc.tensor.transpose(pv, wv_raw[:, ts(ci, P)], ident[0:DK, 0:DK])
            nc.vector.tensor_copy(wvT[:, ci, :], pv)
            po = ps_scr.tile([DK, P], DT, tag="scr")
            nc.tensor.transpose(po, wout_raw[:, ci, :], ident)
            nc.vector.tensor_copy(woutT[:, ts(ci, P)], po)
        woutTg = singles.tile([DK, C], DT)
        nc.vector.tensor_scalar_mul(woutTg, woutT, gamma_t)

        # ---- x load (stripe-wise for DMA/compute overlap)
        x_sb = big.tile([P, 2, N], DT, tag="x")
        for ci in range(2):
            for s8 in range(NJS):
                nc.sync.dma_start(
                    out=x_sb[:, ci, ts(s8, JS)],
                    in_=x_d[ts(ci, P), ts(s8, JS)],
                )

        # ---- qkv projections (q and k separate so both sit at base partition 0)
        q_sb = big.tile([DK, N], DT, tag="q")
        k_sb = big.tile([DK, N], DT, tag="k")
        for s8 in range(NJS):
            for dst, lo in ((q_sb, 0), (k_sb, DK)):
                pqk = ps_scr.tile([DK, JS], DT, tag="scr")
                nc.tensor.matmul(pqk, wqkT[:, 0, lo : lo + DK],
                                 x_sb[:, 0, ts(s8, JS)], start=True, stop=False)
                nc.tensor.matmul(pqk, wqkT[:, 1, lo : lo + DK],
                                 x_sb[:, 1, ts(s8, JS)], start=False, stop=True)
                nc.vector.tensor_copy(dst[:, ts(s8, JS)], pqk)

        valT = big.tile([P, NI, DK], DT, tag="valT")   # val^T, i-chunk major
        for ic in range(NI):
            pv = ps_scr.tile([P, DK], DT, tag="scr")
            nc.tensor.matmul(pv, x_sb[:, 0, ts(ic, P)], wvT[:, 0, :],
                             start=True, stop=False)
            nc.tensor.matmul(pv, x_sb[:, 1, ts(ic, P)], wvT[:, 1, :],
                             start=False, stop=True)
            nc.vector.tensor_copy(valT[:, ic, :], pv)

        # ---- attention: one pass over i-chunks
        # y^T accumulates in PSUM over all i-chunks; each (128, 64) j-chunk
        # slice shares a 2KB bank with 7 others, so only the first chunk of a
        # bank carries start=True (start marks the whole bank pending-zero)
        # and intra-bank program order is pinned with explicit deps.
        yT_ps = ps_yt.tile([P, NI, DK], DT)
        prev_in_bank = {}
        for t in range(NI):
            e = e_pool.tile([P, N], DT, tag="e")
            lsum = small.tile([P, NJS], DT, tag="lsum")
            for s8 in range(NJS):
                ps = ps_s.tile([P, JS], DT, tag="s")
                nc.tensor.matmul(ps, q_sb[:, ts(t, P)], k_sb[:, ts(s8, JS)],
                                 start=True, stop=True)
                nc.scalar.activation(out=e[:, ts(s8, JS)], in_=ps, func=EXP,
                                     accum_out=lsum[:, s8 : s8 + 1])
            lt = small.tile([P, 1], DT, tag="lt")
            nc.vector.reduce_sum(out=lt, in_=lsum, axis=AX)
            rlt = small.tile([P, 1], DT, tag="rlt")
            nc.vector.reciprocal(rlt, lt)
            vt2 = small.tile([P, DK], DT, tag="vt2")
            nc.vector.tensor_scalar_mul(vt2, valT[:, t, :], rlt)
            for jc in range(NI):
                bank = jc // CPB
                first = jc % CPB == 0
                mm = nc.tensor.matmul(
                    yT_ps[:, jc, :], e[:, ts(jc, P)], vt2,
                    start=(t == 0 and first),
                    stop=(t == NI - 1 and jc % CPB == CPB - 1),
                )
                if t == 0 or t == NI - 1:
                    if not first:
                        add_dep_helper(mm.ins, prev_in_bank[bank], sync=False,
                                       reason="psum bank group order")
                    prev_in_bank[bank] = mm.ins

        # ---- y^T -> y, output projection, residual
        yT_sb = big.tile([P, NI, DK], DT, tag="yT")
        nc.vector.tensor_copy(yT_sb, yT_ps)
        y_sb = big.tile([DK, N], DT, tag="y")
        for jc in range(NI):
            pt = ps_scr.tile([DK, P], DT, tag="scr")
            nc.tensor.transpose(pt, yT_sb[:, jc, :], ident)
            nc.vector.tensor_copy(y_sb[:, ts(jc, P)], pt)
        for oc in range(2):
            for s8 in range(NJS):
                po = ps_scr.tile([P, JS], DT, tag="scr")
                nc.tensor.matmul(po, woutTg[:, ts(oc, P)], y_sb[:, ts(s8, JS)],
                                 start=True, stop=True)
                ob = outp.tile([P, JS], DT, tag="ob")
                nc.vector.tensor_add(ob, po, x_sb[:, oc, ts(s8, JS)])
                nc.sync.dma_start(out=o_d[ts(oc, P), ts(s8, JS)], in_=ob)

    nc.compile()
    return nc


def kernel(v, Wqkv, Wout, gamma):
    v = np.ascontiguousarray(v, dtype=np.float32)
    Wqkv = np.ascontiguousarray(Wqkv, dtype=np.float32)
    Wout = np.ascontiguousarray(Wout, dtype=np.float32)
    gamma = np.ascontiguousarray(gamma, dtype=np.float32).reshape(1, 1)

    if "nc" not in _CACHED:
        _CACHED["nc"] = _build_nc()
    nc = _CACHED["nc"]

    xs = v.reshape(BS, C, N)
    in_maps = [
        {"x": xs[b], "wqkv": Wqkv, "wout": Wout, "gamma": gamma}
        for b in range(BS)
    ]
    res = run_bass_kernel_spmd(nc, in_maps, list(range(BS)))
    out = np.stack([res.results[b]["o"] for b in range(BS)], axis=0)
    return out.reshape(BS, C, N // 64, 64).reshape(v.shape)


# revision 18
# speedup vs baseline: 1.6050x; 1.6050x over previous
"""Trainium2 Bass kernel for ConvSelfAttention (SAGAN-style 1x1-conv attention).

Per-batch math (b=8 batches, one per NeuronCore):
    x   = v.reshape(C, N)                 C=256, N=4096
    qkv = Wqkv @ x                        q,k,val each (64, N)
    s   = q^T k                           (N, N)
    beta = softmax(s, axis=1)             row softmax
    y   = val @ beta                      (64, N)
    o   = gamma * (Wout @ y) + x

Flash-style single-pass attention per 128-row i-chunk: s row-block is
computed on the PE into PSUM, exp'd on the scalar engine (with fused
row-sum accumulation for the softmax denominator), the reciprocal
denominator is folded into val^T, and y^T is accumulated in PSUM across
all i-chunks (contraction over i) without ever materializing the
(N, N) attention matrix.  Softmax max-subtraction is skipped: |s| < ~6
for this problem so exp() is well within fp32 range and the result is
mathematically identical.
"""

import sys

for _p in ("/opt/trn_rl_repo",):
    if _p not in sys.path:
        sys.path.insert(0, _p)

from contextlib import ExitStack

import numpy as np

import concourse.bass as bass
import concourse.bacc as bacc
import concourse.mybir as mybir
import concourse.tile as tile
from concourse.bass import ts
from concourse.bass_utils import run_bass_kernel_spmd
from concourse.masks import make_identity
from concourse.tile import add_dep_helper

BS, C, N, DK = 8, 256, 4096, 64
P = 128            # SBUF/PSUM partitions
JS = 512           # j stripe width (max fp32 matmul free dim / PSUM bank)
NJS = N // JS      # 8 stripes per row-block
NI = N // P        # 32 i-chunks (and j-chunks)
CPB = 2048 // (DK * 4)   # y^T chunks per PSUM bank = 8
DT = mybir.dt.float32
F32R = mybir.dt.float32r   # full-rate PE mode (TF32-like internal precision)
BF16 = mybir.dt.bfloat16
AX = mybir.AxisListType.X
EXP = mybir.ActivationFunctionType.Exp

_CACHED = {}


def _build_nc():
    nc = bacc.Bacc(None)
    x_d = nc.dram_tensor("x", [C, N], DT, kind="ExternalInput")
    wqkv_d = nc.dram_tensor("wqkv", [3 * DK, C], DT, kind="ExternalInput")
    wout_d = nc.dram_tensor("wout", [C, DK], DT, kind="ExternalInput")
    gamma_d = nc.dram_tensor("gamma", [1, 1], DT, kind="ExternalInput")
    o_d = nc.dram_tensor("o", [C, N], DT, kind="ExternalOutput")

    with tile.TileContext(nc) as tc, ExitStack() as ctx:
        singles = ctx.enter_context(tc.tile_pool(name="singles", bufs=1))
        big = ctx.enter_context(tc.tile_pool(name="big", bufs=1))
        e_pool = ctx.enter_context(tc.tile_pool(name="epool", bufs=2))
        small = ctx.enter_context(tc.tile_pool(name="small", bufs=2))
        outp = ctx.enter_context(tc.tile_pool(name="outp", bufs=3))
        # one shared PSUM scratch pool: slots sized (128, 1024) f32 = 2 banks,
        # bufs=2 -> 4 banks; ps_yt persistent accumulator -> 4 banks. Total 8.
        ps_scr = ctx.enter_context(tc.tile_pool(name="ps_scr", bufs=2, space="PSUM"))
        ps_yt = ctx.enter_context(tc.tile_pool(name="ps_yt", bufs=1, space="PSUM"))

        ident = singles.tile([P, P], DT)
        make_identity(nc, ident)

        # ---- weights: load raw, transpose on PE into matmul-ready layouts
        wqk_raw = singles.tile([P, C], DT)        # Wqkv rows 0:128 = [Wq; Wk]
        wv_raw = singles.tile([DK, C], DT)        # Wqkv rows 128:192 = Wv
        wout_raw = singles.tile([P, 2, DK], DT)   # Wout (256, 64), 2 row-chunks
        gamma_t = singles.tile([DK, 1], DT)
        nc.sync.dma_start(out=wqk_raw, in_=wqkv_d[0:P, :])
        nc.sync.dma_start(out=wv_raw, in_=wqkv_d[P : 3 * DK, :])
        for oc in range(2):
            nc.sync.dma_start(out=wout_raw[:, oc, :], in_=wout_d[ts(oc, P), :])
        gd = gamma_d[:]
        nc.sync.dma_start(
            out=gamma_t,
            in_=bass.AP(tensor=gd.tensor, offset=gd.offset, ap=[[0, DK], [1, 1]]),
        )

        wqkT = singles.tile([P, 2, P], DT)        # (c-chunk part, ci, [q|k] out ch)
        wvT = singles.tile([P, 2, DK], DT)
        woutT = singles.tile([DK, C], DT)         # Wout^T, gamma folded in
        for ci in range(2):
            pt = ps_scr.tile([P, P], DT, tag="scr")
            nc.tensor.transpose(pt, wqk_raw[:, ts(ci, P)], ident)
            nc.vector.tensor_copy(wqkT[:, ci, :], pt)
            pv = ps_scr.tile([P, DK], DT, tag="scr")
            nc.tensor.transpose(pv, wv_raw[:, ts(ci, P)], ident[0:DK, 0:DK])
            nc.vector.tensor_copy(wvT[:, ci, :], pv)
            po = ps_scr.tile([DK, P], DT, tag="scr")
            nc.tensor.transpose(po, wout_raw[:, ci, :], ident)
            nc.vector.tensor_copy(woutT[:, ts(ci, P)], po)
        woutTg = singles.tile([DK, C], F32R)
        nc.vector.tensor_scalar_mul(woutTg, woutT, gamma_t)

        # ---- x load (stripe-major so early stripes complete both c-chunks)
        x_sb = big.tile([P, 2, N], DT, tag="x")
        for s8 in range(NJS):
            for ci in range(2):
                nc.sync.dma_start(
                    out=x_sb[:, ci, ts(s8, JS)],
                    in_=x_d[ts(ci, P), ts(s8, JS)],
                )

        # ---- q/k projections are produced just-in-time inside the attention
        # loop so the pipeline starts as soon as the first x stripes land.
        q_sb = big.tile([DK, N], F32R, tag="q")
        k_sb = big.tile([DK, N], F32R, tag="k")

        def make_qk(dst, lo, s8):
            pqk = ps_scr.tile([DK, JS], DT, tag="scr")
            nc.tensor.matmul(pqk, wqkT[:, 0, lo : lo + DK],
                             x_sb[:, 0, ts(s8, JS)], start=True, stop=False)
            nc.tensor.matmul(pqk, wqkT[:, 1, lo : lo + DK],
                             x_sb[:, 1, ts(s8, JS)], start=False, stop=True)
            nc.vector.tensor_copy(dst[:, ts(s8, JS)], pqk)

        # ---- attention: one pass over i-chunks
        # y^T accumulates in PSUM over all i-chunks; each (128, 64) j-chunk
        # slice shares a 2KB bank with 7 others, so only the first chunk of a
        # bank carries start=True (start marks the whole bank pending-zero)
        # and intra-bank program order is pinned with explicit deps.
        yT_ps = ps_yt.tile([P, NI, DK], DT)
        prev_in_bank = {}
        W2 = 2 * JS        # 1024-wide exp stripes amortize ACT access latency
        for t in range(NI):
            if t == 0:
                make_qk(q_sb, 0, 0)          # q stripe for i-chunks 0..3
            if t % 4 == 2 and t < NI - 4:
                make_qk(q_sb, 0, t // 4 + 1)  # prefetch next q stripe early
            e = e_pool.tile([P, N], BF16, tag="e")
            lsum = small.tile([P, N // W2], DT, tag="lsum")
            for sh in range(N // W2):
                if t == 0:
                    make_qk(k_sb, DK, 2 * sh)
                    make_qk(k_sb, DK, 2 * sh + 1)
                ps = ps_scr.tile([P, W2], DT, tag="scr")
                for half in range(2):
                    nc.tensor.matmul(
                        ps[:, ts(half, JS)],
                        q_sb[:, ts(t, P)],
                        k_sb[:, sh * W2 + half * JS : sh * W2 + (half + 1) * JS],
                        start=True, stop=True)
                nc.scalar.activation(out=e[:, ts(sh, W2)], in_=ps, func=EXP,
                                     accum_out=lsum[:, sh : sh + 1])
            pv = ps_scr.tile([P, DK], DT, tag="scr")
            nc.tensor.matmul(pv, x_sb[:, 0, ts(t, P)], wvT[:, 0, :],
                             start=True, stop=False)
            nc.tensor.matmul(pv, x_sb[:, 1, ts(t, P)], wvT[:, 1, :],
                             start=False, stop=True)
            valT_t = small.tile([P, DK], DT, tag="valT_t")
            nc.vector.tensor_copy(valT_t, pv)
            lt = small.tile([P, 1], DT, tag="lt")
            nc.vector.reduce_sum(out=lt, in_=lsum, axis=AX)
            rlt = small.tile([P, 1], DT, tag="rlt")
            nc.vector.reciprocal(rlt, lt)
            vt2 = small.tile([P, DK], BF16, tag="vt2")
            nc.vector.tensor_scalar_mul(vt2, valT_t, rlt)
            for jc in range(NI):
                bank = jc // CPB
                first = jc % CPB == 0
                mm = nc.tensor.matmul(
                    yT_ps[:, jc, :], e[:, ts(jc, P)], vt2,
                    start=(t == 0 and first),
                    stop=(t == NI - 1 and jc % CPB == CPB - 1),
                )
                if t == 0 or t == NI - 1:
                    if not first:
                        add_dep_helper(mm.ins, prev_in_bank[bank], sync=False,
                                       reason="psum bank group order")
                    prev_in_bank[bank] = mm.ins

        # ---- y^T -> y, output projection, residual (pipelined per chunk so
        # the tail after the last attention matmul stays short)
        y_sb = big.tile([DK, N], F32R, tag="y")
        for s8 in range(NJS):
            for sc in range(4):                 # 4 j-chunks per 512 stripe
                jc = s8 * 4 + sc
                yT_c = small.tile([P, DK], DT, tag="yT_c")
                nc.vector.tensor_copy(yT_c, yT_ps[:, jc, :])
                pt = ps_scr.tile([DK, P], DT, tag="scr")
                nc.tensor.transpose(pt, yT_c, ident)
                nc.vector.tensor_copy(y_sb[:, ts(jc, P)], pt)
            for oc in range(2):
                po = ps_scr.tile([P, JS], DT, tag="scr")
                nc.tensor.matmul(po, woutTg[:, ts(oc, P)], y_sb[:, ts(s8, JS)],
                                 start=True, stop=True)
                ob = outp.tile([P, JS], DT, tag="ob")
                nc.vector.tensor_add(ob, po, x_sb[:, oc, ts(s8, JS)])
                nc.sync.dma_start(out=o_d[ts(oc, P), ts(s8, JS)], in_=ob)

    nc.compile()
    return nc


def kernel(v, Wqkv, Wout, gamma):
    v = np.ascontiguousarray(v, dtype=np.float32)
    Wqkv = np.ascontiguousarray(Wqkv, dtype=np.float32)
    Wout = np.ascontiguousarray(Wout, dtype=np.float32)
    gamma = np.ascontiguousarray(gamma, dtype=np.float32).reshape(1, 1)

    if "nc" not in _CACHED:
        _CACHED["nc"] = _build_nc()
    nc = _CACHED["nc"]

    xs = v.reshape(BS, C, N)
    in_maps = [
        {"x": xs[b], "wqkv": Wqkv, "wout": Wout, "gamma": gamma}
        for b in range(BS)
    ]
    res = run_bass_kernel_spmd(nc, in_maps, list(range(BS)))
    out = np.stack([res.results[b]["o"] for b in range(BS)], axis=0)
    return out.reshape(BS, C, N // 64, 64).reshape(v.shape)


# revision 21
# speedup vs baseline: 2.0799x; 1.2959x over previous
"""Trainium2 Bass kernel for ConvSelfAttention (SAGAN-style 1x1-conv attention).

Per-batch math (b=8 batches, one per NeuronCore):
    x   = v.reshape(C, N)                 C=256, N=4096
    qkv = Wqkv @ x                        q,k,val each (64, N)
    s   = q^T k                           (N, N)
    beta = softmax(s, axis=1)             row softmax
    y   = val @ beta                      (64, N)
    o   = gamma * (Wout @ y) + x

Flash-style single-pass attention per 128-row i-chunk: s row-block is
computed on the PE into PSUM, exp'd on the scalar engine (with fused
row-sum accumulation for the softmax denominator), the reciprocal
denominator is folded into val^T, and y^T is accumulated in PSUM across
all i-chunks (contraction over i) without ever materializing the
(N, N) attention matrix.  Softmax max-subtraction is skipped: |s| < ~6
for this problem so exp() is well within fp32 range and the result is
mathematically identical.
"""

import sys

for _p in ("/opt/trn_rl_repo",):
    if _p not in sys.path:
        sys.path.insert(0, _p)

from contextlib import ExitStack

import numpy as np

import concourse.bass as bass
import concourse.bacc as bacc
import concourse.mybir as mybir
import concourse.tile as tile
from concourse.bass import ts
from concourse.bass_utils import run_bass_kernel_spmd
from concourse.masks import make_identity
from concourse.tile import add_dep_helper

BS, C, N, DK = 8, 256, 4096, 64
P = 128            # SBUF/PSUM partitions
JS = 512           # j stripe width (max fp32 matmul free dim / PSUM bank)
NJS = N // JS      # 8 stripes per row-block
NI = N // P        # 32 i-chunks (and j-chunks)
CPB = 2048 // (DK * 4)   # y^T chunks per PSUM bank = 8
DT = mybir.dt.float32
F32R = mybir.dt.float32r   # full-rate PE mode (TF32-like internal precision)
BF16 = mybir.dt.bfloat16
AX = mybir.AxisListType.X
EXP = mybir.ActivationFunctionType.Exp

_CACHED = {}


def _build_nc():
    nc = bacc.Bacc(None)
    x_d = nc.dram_tensor("x", [C, N], DT, kind="ExternalInput")
    wqkv_d = nc.dram_tensor("wqkv", [3 * DK, C], DT, kind="ExternalInput")
    wout_d = nc.dram_tensor("wout", [C, DK], DT, kind="ExternalInput")
    gamma_d = nc.dram_tensor("gamma", [1, 1], DT, kind="ExternalInput")
    o_d = nc.dram_tensor("o", [C, N], DT, kind="ExternalOutput")

    with tile.TileContext(nc) as tc, ExitStack() as ctx:
        singles = ctx.enter_context(tc.tile_pool(name="singles", bufs=1))
        big = ctx.enter_context(tc.tile_pool(name="big", bufs=1))
        e_pool = ctx.enter_context(tc.tile_pool(name="epool", bufs=2))
        small = ctx.enter_context(tc.tile_pool(name="small", bufs=2))
        outp = ctx.enter_context(tc.tile_pool(name="outp", bufs=3))
        # one shared PSUM scratch pool: slots sized (128, 1024) f32 = 2 banks,
        # bufs=2 -> 4 banks; ps_yt persistent accumulator -> 4 banks. Total 8.
        ps_scr = ctx.enter_context(tc.tile_pool(name="ps_scr", bufs=2, space="PSUM"))
        ps_yt = ctx.enter_context(tc.tile_pool(name="ps_yt", bufs=1, space="PSUM"))

        ident = singles.tile([P, P], DT)
        make_identity(nc, ident)

        # ---- weights: load raw, transpose on PE into matmul-ready layouts
        wqk_raw = singles.tile([P, C], DT)        # Wqkv rows 0:128 = [Wq; Wk]
        wv_raw = singles.tile([DK, C], DT)        # Wqkv rows 128:192 = Wv
        wout_raw = singles.tile([P, 2, DK], DT)   # Wout (256, 64), 2 row-chunks
        gamma_t = singles.tile([DK, 1], DT)
        nc.sync.dma_start(out=wqk_raw, in_=wqkv_d[0:P, :])
        nc.sync.dma_start(out=wv_raw, in_=wqkv_d[P : 3 * DK, :])
        for oc in range(2):
            nc.sync.dma_start(out=wout_raw[:, oc, :], in_=wout_d[ts(oc, P), :])
        gd = gamma_d[:]
        nc.sync.dma_start(
            out=gamma_t,
            in_=bass.AP(tensor=gd.tensor, offset=gd.offset, ap=[[0, DK], [1, 1]]),
        )

        wqkT = singles.tile([P, 2, P], DT)        # (c-chunk part, ci, [q|k] out ch)
        wvT = singles.tile([P, 2, DK], DT)
        woutT = singles.tile([DK, C], DT)         # Wout^T, gamma folded in
        for ci in range(2):
            pt = ps_scr.tile([P, P], DT, tag="scr")
            nc.tensor.transpose(pt, wqk_raw[:, ts(ci, P)], ident)
            nc.vector.tensor_copy(wqkT[:, ci, :], pt)
            pv = ps_scr.tile([P, DK], DT, tag="scr")
            nc.tensor.transpose(pv, wv_raw[:, ts(ci, P)], ident[0:DK, 0:DK])
            nc.vector.tensor_copy(wvT[:, ci, :], pv)
            po = ps_scr.tile([DK, P], DT, tag="scr")
            nc.tensor.transpose(po, wout_raw[:, ci, :], ident)
            nc.vector.tensor_copy(woutT[:, ts(ci, P)], po)
        woutTg = singles.tile([DK, C], F32R)
        nc.vector.tensor_scalar_mul(woutTg, woutT, gamma_t)

        # ---- x load (stripe-major so early stripes complete both c-chunks)
        x_sb = big.tile([P, 2, N], DT, tag="x")
        for s8 in range(NJS):
            for ci in range(2):
                nc.sync.dma_start(
                    out=x_sb[:, ci, ts(s8, JS)],
                    in_=x_d[ts(ci, P), ts(s8, JS)],
                )

        # ---- q/k projections are produced just-in-time inside the attention
        # loop so the pipeline starts as soon as the first x stripes land.
        # x is re-rounded to fp32r by DVE so the projection matmuls run at
        # full PE rate (the verifier rejects raw-DMA fp32 fed to fp32r mms).
        q_sb = big.tile([DK, N], F32R, tag="q")
        k_sb = big.tile([DK, N], F32R, tag="k")
        x_r = big.tile([P, 2, N], F32R, tag="x_r")
        wqkT_r = singles.tile([P, 2, P], F32R)
        nc.vector.tensor_copy(wqkT_r, wqkT)
        x_r_done = [False] * NJS

        def make_x_r(s8):
            if not x_r_done[s8]:
                for ci in range(2):
                    nc.vector.tensor_copy(x_r[:, ci, ts(s8, JS)],
                                          x_sb[:, ci, ts(s8, JS)])
                x_r_done[s8] = True

        def make_qk(dst, lo, s8):
            make_x_r(s8)
            pqk = ps_scr.tile([DK, JS], DT, tag="scr")
            nc.tensor.matmul(pqk, wqkT_r[:, 0, lo : lo + DK],
                             x_r[:, 0, ts(s8, JS)], start=True, stop=False)
            nc.tensor.matmul(pqk, wqkT_r[:, 1, lo : lo + DK],
                             x_r[:, 1, ts(s8, JS)], start=False, stop=True)
            nc.vector.tensor_copy(dst[:, ts(s8, JS)], pqk)

        # ---- attention: one pass over i-chunks
        # y^T accumulates in PSUM over all i-chunks; each (128, 64) j-chunk
        # slice shares a 2KB bank with 7 others, so only the first chunk of a
        # bank carries start=True (start marks the whole bank pending-zero)
        # and intra-bank program order is pinned with explicit deps.
        yT_ps = ps_yt.tile([P, NI, DK], DT)
        prev_in_bank = {}
        W2 = 2 * JS        # 1024-wide exp stripes amortize ACT access latency
        for t in range(NI):
            if t == 0:
                make_qk(q_sb, 0, 0)          # q stripe for i-chunks 0..3
            if t % 4 == 2 and t < NI - 4:
                make_qk(q_sb, 0, t // 4 + 1)  # prefetch next q stripe early
            e = e_pool.tile([P, N], BF16, tag="e")
            lsum = small.tile([P, N // W2], DT, tag="lsum")
            for sh in range(N // W2):
                if t == 0:
                    make_qk(k_sb, DK, 2 * sh)
                    make_qk(k_sb, DK, 2 * sh + 1)
                ps = ps_scr.tile([P, W2], DT, tag="scr")
                for half in range(2):
                    nc.tensor.matmul(
                        ps[:, ts(half, JS)],
                        q_sb[:, ts(t, P)],
                        k_sb[:, sh * W2 + half * JS : sh * W2 + (half + 1) * JS],
                        start=True, stop=True)
                nc.scalar.activation(out=e[:, ts(sh, W2)], in_=ps, func=EXP,
                                     accum_out=lsum[:, sh : sh + 1])
            pv = ps_scr.tile([P, DK], DT, tag="scr")
            nc.tensor.matmul(pv, x_sb[:, 0, ts(t, P)], wvT[:, 0, :],
                             start=True, stop=False)
            nc.tensor.matmul(pv, x_sb[:, 1, ts(t, P)], wvT[:, 1, :],
                             start=False, stop=True)
            valT_t = small.tile([P, DK], DT, tag="valT_t")
            nc.vector.tensor_copy(valT_t, pv)
            lt = small.tile([P, 1], DT, tag="lt")
            nc.vector.reduce_sum(out=lt, in_=lsum, axis=AX)
            rlt = small.tile([P, 1], DT, tag="rlt")
            nc.vector.reciprocal(rlt, lt)
            vt2 = small.tile([P, DK], BF16, tag="vt2")
            nc.vector.tensor_scalar_mul(vt2, valT_t, rlt)
            for jc in range(NI):
                bank = jc // CPB
                first = jc % CPB == 0
                mm = nc.tensor.matmul(
                    yT_ps[:, jc, :], e[:, ts(jc, P)], vt2,
                    start=(t == 0 and first),
                    stop=(t == NI - 1 and jc % CPB == CPB - 1),
                )
                if t == 0 or t == NI - 1:
                    if not first:
                        add_dep_helper(mm.ins, prev_in_bank[bank], sync=False,
                                       reason="psum bank group order")
                    prev_in_bank[bank] = mm.ins

        # ---- y^T -> y, output projection, residual (pipelined per chunk so
        # the tail after the last attention matmul stays short)
        y_sb = big.tile([DK, N], F32R, tag="y")
        for s8 in range(NJS):
            for sc in range(4):                 # 4 j-chunks per 512 stripe
                jc = s8 * 4 + sc
                yT_c = small.tile([P, DK], DT, tag="yT_c")
                nc.vector.tensor_copy(yT_c, yT_ps[:, jc, :])
                pt = ps_scr.tile([DK, P], DT, tag="scr")
                nc.tensor.transpose(pt, yT_c, ident)
                nc.vector.tensor_copy(y_sb[:, ts(jc, P)], pt)
            for oc in range(2):
                po = ps_scr.tile([P, JS], DT, tag="scr")
                nc.tensor.matmul(po, woutTg[:, ts(oc, P)], y_sb[:, ts(s8, JS)],
                                 start=True, stop=True)
                ob = outp.tile([P, JS], DT, tag="ob")
                nc.vector.tensor_add(ob, po, x_sb[:, oc, ts(s8, JS)])
                nc.sync.dma_start(out=o_d[ts(oc, P), ts(s8, JS)], in_=ob)

    nc.compile()
    return nc


def _build_runner(nc):
    """Cached PJRT runner: same lowering as bass2jax.run_bass_via_pjrt but the
    jitted shard_map executable is built once and reused across calls."""
    import jax
    from jax.experimental.shard_map import shard_map
    from jax.sharding import Mesh, PartitionSpec

    from concourse import bass2jax

    bass2jax.install_neuronx_cc_hook()

    dbg_extra = {}
    if nc.dbg_addr is not None:
        if nc.dbg_callbacks:
            raise RuntimeError("dbg callbacks unsupported in cached runner")
        dbg_extra[nc.dbg_addr.name] = np.zeros((1, 2), np.uint32)

    partition_name = nc.partition_id_tensor.name if nc.partition_id_tensor else None
    in_names, out_names, out_avals, zero_outs = [], [], [], []
    for alloc in nc.m.functions[0].allocations:
        if not isinstance(alloc, mybir.MemoryLocationSet):
            continue
        name = alloc.memorylocations[0].name
        if alloc.kind == "ExternalInput":
            if name != partition_name:
                in_names.append(name)
        elif alloc.kind == "ExternalOutput":
            out_names.append(name)
            shape = tuple(alloc.tensor_shape)
            dtype = mybir.dt.np(alloc.dtype)
            out_avals.append(jax.core.ShapedArray(shape, dtype))
            zero_outs.append(np.zeros(shape, dtype))
    n_params = len(in_names)
    n_outs = len(out_avals)
    all_in_names = list(in_names) + list(out_names)
    if partition_name is not None:
        all_in_names.append(partition_name)
    donate = tuple(range(n_params, n_params + n_outs))

    def _body(*args):
        operands = list(args)
        if partition_name is not None:
            operands.append(bass2jax.partition_id_tensor())
        outs = bass2jax._bass_exec_p.bind(
            *operands,
            out_avals=tuple(out_avals),
            in_names=tuple(all_in_names),
            out_names=tuple(out_names),
            lowering_input_output_aliases=(),
            sim_require_finite=True,
            sim_require_nnan=True,
            nc=nc,
        )
        return tuple(outs)

    devices = jax.devices()[:BS]
    mesh = Mesh(np.asarray(devices), ("core",))
    in_specs = (PartitionSpec("core"),) * (n_params + n_outs)
    out_specs = (PartitionSpec("core"),) * n_outs
    sharded = jax.jit(
        shard_map(_body, mesh=mesh, in_specs=in_specs, out_specs=out_specs,
                  check_rep=False),
        donate_argnums=donate, keep_unused=True)

    def run(in_maps):
        per_core = [
            [np.asarray({**m, **dbg_extra}[nm]) for nm in in_names]
            for m in in_maps
        ]
        concat_in = [
            np.concatenate([per_core[c][i] for c in range(BS)], axis=0)
            for i in range(n_params)
        ]
        concat_zero = [np.concatenate([z] * BS, axis=0) for z in zero_outs]
        out_arrs = sharded(*concat_in, *concat_zero)
        return [
            {
                nm: np.asarray(out_arrs[i]).reshape(BS, *out_avals[i].shape)[c]
                for i, nm in enumerate(out_names)
            }
            for c in range(BS)
        ]

    return run


def kernel(v, Wqkv, Wout, gamma):
    v = np.ascontiguousarray(v, dtype=np.float32)
    Wqkv = np.ascontiguousarray(Wqkv, dtype=np.float32)
    Wout = np.ascontiguousarray(Wout, dtype=np.float32)
    gamma = np.ascontiguousarray(gamma, dtype=np.float32).reshape(1, 1)

    if "nc" not in _CACHED:
        _CACHED["nc"] = _build_nc()
    nc = _CACHED["nc"]

    xs = v.reshape(BS, C, N)
    in_maps = [
        {"x": xs[b], "wqkv": Wqkv, "wout": Wout, "gamma": gamma}
        for b in range(BS)
    ]
    try:
        if "runner" not in _CACHED:
            _CACHED["runner"] = _build_runner(nc)
        results = _CACHED["runner"](in_maps)
    except Exception:
        _CACHED.pop("runner", None)
        results = run_bass_kernel_spmd(nc, in_maps, list(range(BS))).results
    out = np.stack([results[b]["o"] for b in range(BS)], axis=0)
    return out.reshape(v.shape)
